# revision 10
# baseline (speedup 1.0000x reference)
"""Trainium2 Bass kernel for IrrepsLinear (128x0e + 128x1o + 128x2e).

y[n, off_l + o*d_l + d] = alpha * sum_m x[n, off_l + m*d_l + d] * W_l[m, o]

Data-parallel over nodes N across 8 cores; the whole data path runs in fp16
(fp32 accumulation in PSUM) — the harness gate is rel_err < 2e-2 and fp16
keeps it ~5e-4, while halving HBM traffic vs fp32 (the fp32 version sat at
the 358 GB/s HBM roofline).

Host-side sharding lays each core's x shard out m-major as
xg[128, 49, 9, 128] fp16: partition line m holds, for each 128-node subtile,
nine de-interleaved plane rows (one per (l, d) pair). A chunk of subtiles is
then a single contiguous DMA run per partition (up to ~16 KB descriptors).
Per subtile nine fp16 matmuls (lhsT = plane slice, rhs = alpha-scaled weight
resident in SBUF) fill two PSUM tiles: P1 [128,512] = l2 d0-3 (one bank) and
P2 [128,640] = l2 d4, l0, l1 d0-2 (two banks). One DVE copy (P1) and one ACT
copy (P2) cast fp32 -> fp16 into the plane-major SBUF output tile, which is
DMA'd to the m-major output y[128, 49, 1152]; the host inverse-permutes.
Input DMAs ride the SP HWDGE ring, output DMAs the ACT ring. Chunk sizes
taper ([3,7,...,7,3,1]) so compute starts early and the tail drains fast.
"""

import sys

sys.path.insert(0, "/opt/trn_rl_repo")

import numpy as np

N = 50000
FEAT = 1152
DIMS = [1, 3, 5]
OFFS = [0, 128, 512]
N_CORES = 8
SUB = 128            # nodes per subtile (partition dim)
NSUB = 49            # subtiles per core
NPC = NSUB * SUB     # padded nodes per core (6272)
SIZES = [1, 2, 4, 7, 7, 7, 7, 7, 6, 1]   # subtiles per DMA unit (sum = 49)
CHMAX = max(SIZES)

# (l, d) plane order, both for the xg input and the plane-major output:
# l2 d0-4 first, then l0, then l1 d0-2.
PLANES = [(2, 0), (2, 1), (2, 2), (2, 3), (2, 4), (0, 0), (1, 0), (1, 1),
          (1, 2)]

_COMPILED = None


def build_nc(sizes=tuple(SIZES)):
    import concourse.mybir as mybir
    import concourse.tile as tile
    from concourse import bacc

    f16 = mybir.dt.float16
    f32 = mybir.dt.float32
    nsub = sum(sizes)

    nc = bacc.Bacc("TRN2", target_bir_lowering=False, debug=False,
                   num_devices=N_CORES)
    xg = nc.dram_tensor("xg", [128, nsub, 9, SUB], f16, kind="ExternalInput")
    w = nc.dram_tensor("w", [128, 3, 128], f16, kind="ExternalInput")
    y = nc.dram_tensor("y", [128, nsub, FEAT], f16, kind="ExternalOutput")

    with tile.TileContext(nc) as tc:
        with (
            tc.tile_pool(name="singles", bufs=1) as singles,
            tc.tile_pool(name="xs", bufs=4) as xpool,
            tc.tile_pool(name="ys", bufs=4) as ypool,
            tc.tile_pool(name="p1", bufs=4, space="PSUM") as p1pool,
            tc.tile_pool(name="p2", bufs=2, space="PSUM") as p2pool,
        ):
            wt = singles.tile([128, 3, 128], f16, tag="w")
            nc.sync.dma_start(out=wt, in_=w[:, :, :])
            wts = [wt[:, i, :] for i in range(3)]

            s0 = 0
            for csz in sizes:
                xt = xpool.tile([128, CHMAX, 9, SUB], f16)
                nc.sync.dma_start(out=xt[:, 0:csz], in_=xg[:, s0:s0 + csz])
                yt = ypool.tile([128, CHMAX, FEAT], f16)
                for ai in range(csz):
                    def lhs(q):
                        return xt[:, ai, q, :]

                    # P1: l2 d0-3 (one PSUM bank)
                    p1 = p1pool.tile([128, 512], f32, tag="p1")
                    for d in range(4):
                        nc.tensor.matmul(p1[:, d * 128:(d + 1) * 128],
                                         lhsT=lhs(d), rhs=wts[2])
                    # P2: l2 d4 | l0 | l1 d0-2 (two PSUM banks; every
                    # matmul stays within a single bank)
                    p2 = p2pool.tile([128, 640], f32, tag="p2")
                    nc.tensor.matmul(p2[:, 0:128], lhsT=lhs(4), rhs=wts[2])
                    nc.tensor.matmul(p2[:, 128:256], lhsT=lhs(5), rhs=wts[0])
                    for d in range(3):
                        nc.tensor.matmul(
                            p2[:, (d + 2) * 128:(d + 3) * 128],
                            lhsT=lhs(6 + d), rhs=wts[1])

                    # contiguous PSUM -> SBUF copies (fp32 -> fp16 cast),
                    # plane-major output; host undoes the column permute.
                    yrow = yt[:, ai, :]
                    nc.vector.tensor_copy(yrow[:, 0:512], p1)
                    nc.scalar.copy(yrow[:, 512:1152], p2)
                # output DMA on the ACT HWDGE ring (separate FIFO from inputs)
                nc.scalar.dma_start(out=y[:, s0:s0 + csz], in_=yt[:, 0:csz])
                s0 += csz

    nc.compile()
    return nc


# plane q row m <- original feature column off_l + m*d_l + d; also the
# output-side permutation (plane-major column q*128+o -> natural column).
_PERM = np.concatenate([
    np.arange(128) * DIMS[l] + OFFS[l] + d for (l, d) in PLANES
])
_INV = np.empty(FEAT, np.int64)
_INV[_PERM] = np.arange(FEAT)


def _shard_inputs(x, W0, W1, W2):
    alpha = np.float32(1.0 / np.sqrt(128.0))
    ws = {"w": np.ascontiguousarray(
        np.stack([W0 * alpha, W1 * alpha, W2 * alpha], axis=1),
        dtype=np.float16)}
    x16 = np.asarray(x, dtype=np.float16)
    in_maps = []
    for i in range(N_CORES):
        lo = i * NPC
        hi = min(lo + NPC, N)
        xs = x16[lo:hi]
        xp = np.empty((9 * 128, NPC), np.float16)
        xp[:, : hi - lo] = xs.T[_PERM]
        if hi - lo < NPC:
            xp[:, hi - lo:] = 0.0
        # [9, 128m, nsub, 128n] -> m-major [128m, nsub, 9, 128n]
        xg = np.ascontiguousarray(
            xp.reshape(9, 128, NSUB, SUB).transpose(1, 2, 0, 3))
        in_maps.append({"xg": xg, **ws})
    return in_maps


def _unshard_output(results):
    out = np.empty((N, FEAT), np.float32)
    for i in range(N_CORES):
        lo = i * NPC
        hi = min(lo + NPC, N)
        # y[128p, nsub, feat] -> node-major [nsub*128, feat]
        yp = results[i]["y"].transpose(1, 0, 2).reshape(NPC, FEAT)[: hi - lo]
        out[lo:hi] = yp[:, _INV]
    return out


def kernel(x, W0, W1, W2):
    global _COMPILED
    from concourse.bass_utils import run_bass_kernel_spmd

    if _COMPILED is None:
        _COMPILED = build_nc()
    nc = _COMPILED
    in_maps = _shard_inputs(np.asarray(x), np.asarray(W0), np.asarray(W1),
                            np.asarray(W2))
    res = run_bass_kernel_spmd(nc, in_maps, list(range(N_CORES)))
    return _unshard_output(res.results)


# revision 13
# speedup vs baseline: 1.0123x; 1.0123x over previous
"""Trainium2 Bass kernel for IrrepsLinear (128x0e + 128x1o + 128x2e).

y[n, off_l + o*d_l + d] = alpha * sum_m x[n, off_l + m*d_l + d] * W_l[m, o]

Data-parallel over nodes N across 8 cores; the whole data path runs in fp16
(fp32 accumulation in PSUM) — the harness gate is rel_err < 2e-2 and fp16
keeps it ~5e-4, while halving HBM traffic vs fp32 (the fp32 version sat at
the 358 GB/s HBM roofline).

Host-side sharding lays each core's x shard out m-major as
xg[128, 49, 9, 128] fp16: partition line m holds, for each 128-node subtile,
nine de-interleaved plane rows (one per (l, d) pair). A chunk of subtiles is
then a single contiguous DMA run per partition (up to ~16 KB descriptors).
Per subtile nine fp16 matmuls (lhsT = plane slice, rhs = alpha-scaled weight
resident in SBUF) fill two PSUM tiles: P1 [128,512] = l2 d0-3 (one bank) and
P2 [128,640] = l2 d4, l0, l1 d0-2 (two banks). One DVE copy (P1) and one ACT
copy (P2) cast fp32 -> fp16 into the plane-major SBUF output tile, which is
DMA'd to the m-major output y[128, 49, 1152]; the host inverse-permutes.
Input DMAs ride the SP HWDGE ring, output DMAs the ACT ring. Chunk sizes
taper ([3,7,...,7,3,1]) so compute starts early and the tail drains fast.
"""

import sys

sys.path.insert(0, "/opt/trn_rl_repo")

import numpy as np

N = 50000
FEAT = 1152
DIMS = [1, 3, 5]
OFFS = [0, 128, 512]
N_CORES = 8
SUB = 128            # nodes per subtile (partition dim)
NSUB = 49            # subtiles per core
NPC = NSUB * SUB     # padded nodes per core (6272)
SIZES = [3, 7, 7, 7, 7, 7, 7, 3, 1]   # subtiles per DMA unit (sum = 49)
CHMAX = max(SIZES)

# (l, d) plane order, both for the xg input and the plane-major output:
# l2 d0-4 first, then l0, then l1 d0-2.
PLANES = [(2, 0), (2, 1), (2, 2), (2, 3), (2, 4), (0, 0), (1, 0), (1, 1),
          (1, 2)]

_COMPILED = None


def build_nc(sizes=tuple(SIZES)):
    import concourse.mybir as mybir
    import concourse.tile as tile
    from concourse import bacc

    f16 = mybir.dt.float16
    f32 = mybir.dt.float32
    nsub = sum(sizes)

    nc = bacc.Bacc("TRN2", target_bir_lowering=False, debug=False,
                   num_devices=N_CORES)
    xg = nc.dram_tensor("xg", [128, nsub, 9, SUB], f16, kind="ExternalInput")
    w = nc.dram_tensor("w", [128, 3, 128], f16, kind="ExternalInput")
    y = nc.dram_tensor("y", [128, nsub, FEAT], f16, kind="ExternalOutput")

    with tile.TileContext(nc) as tc:
        with (
            tc.tile_pool(name="singles", bufs=1) as singles,
            tc.tile_pool(name="xs", bufs=4) as xpool,
            tc.tile_pool(name="ys", bufs=4) as ypool,
            tc.tile_pool(name="p1", bufs=2, space="PSUM") as p1pool,
            tc.tile_pool(name="p2", bufs=2, space="PSUM") as p2pool,
        ):
            wt = singles.tile([128, 3, 128], f16, tag="w")
            nc.sync.dma_start(out=wt, in_=w[:, :, :])
            wts = [wt[:, i, :] for i in range(3)]

            s0 = 0
            for csz in sizes:
                xt = xpool.tile([128, CHMAX, 9, SUB], f16)
                nc.sync.dma_start(out=xt[:, 0:csz], in_=xg[:, s0:s0 + csz])
                yt = ypool.tile([128, CHMAX, FEAT], f16)
                for ai in range(0, csz, 2):
                    npair = min(2, csz - ai)

                    def lhs(j, q):
                        return xt[:, ai + j, q, :]

                    # P1 pair: l2 d0-3 for two subtiles (two PSUM banks) —
                    # one scalar copy per pair halves the per-instruction
                    # fixed cost on the copy engines.
                    p1 = p1pool.tile([128, 2, 512], f32, tag="p1")
                    p2s = []
                    for j in range(npair):
                        for d in range(4):
                            nc.tensor.matmul(p1[:, j, d * 128:(d + 1) * 128],
                                             lhsT=lhs(j, d), rhs=wts[2])
                        # P2: l2 d4 | l0 | l1 d0-2 (two PSUM banks; every
                        # matmul stays within a single bank)
                        p2 = p2pool.tile([128, 640], f32, tag="p2")
                        nc.tensor.matmul(p2[:, 0:128], lhsT=lhs(j, 4),
                                         rhs=wts[2])
                        nc.tensor.matmul(p2[:, 128:256], lhsT=lhs(j, 5),
                                         rhs=wts[0])
                        for d in range(3):
                            nc.tensor.matmul(
                                p2[:, (d + 2) * 128:(d + 3) * 128],
                                lhsT=lhs(j, 6 + d), rhs=wts[1])
                        p2s.append(p2)

                    # PSUM -> SBUF copies (fp32 -> fp16 cast), plane-major
                    # output; host undoes the column permute.
                    nc.scalar.copy(yt[:, ai:ai + npair, 0:512],
                                   p1[:, 0:npair])
                    for j in range(npair):
                        nc.vector.tensor_copy(yt[:, ai + j, 512:1152],
                                              p2s[j])
                # output DMA on the ACT HWDGE ring (separate FIFO from inputs)
                nc.scalar.dma_start(out=y[:, s0:s0 + csz], in_=yt[:, 0:csz])
                s0 += csz

    nc.compile()
    return nc


# plane q row m <- original feature column off_l + m*d_l + d; also the
# output-side permutation (plane-major column q*128+o -> natural column).
_PERM = np.concatenate([
    np.arange(128) * DIMS[l] + OFFS[l] + d for (l, d) in PLANES
])
_INV = np.empty(FEAT, np.int64)
_INV[_PERM] = np.arange(FEAT)


def _shard_inputs(x, W0, W1, W2):
    alpha = np.float32(1.0 / np.sqrt(128.0))
    ws = {"w": np.ascontiguousarray(
        np.stack([W0 * alpha, W1 * alpha, W2 * alpha], axis=1),
        dtype=np.float16)}
    x16 = np.asarray(x, dtype=np.float16)
    in_maps = []
    for i in range(N_CORES):
        lo = i * NPC
        hi = min(lo + NPC, N)
        xs = x16[lo:hi]
        xp = np.empty((9 * 128, NPC), np.float16)
        xp[:, : hi - lo] = xs.T[_PERM]
        if hi - lo < NPC:
            xp[:, hi - lo:] = 0.0
        # [9, 128m, nsub, 128n] -> m-major [128m, nsub, 9, 128n]
        xg = np.ascontiguousarray(
            xp.reshape(9, 128, NSUB, SUB).transpose(1, 2, 0, 3))
        in_maps.append({"xg": xg, **ws})
    return in_maps


def _unshard_output(results):
    out = np.empty((N, FEAT), np.float32)
    for i in range(N_CORES):
        lo = i * NPC
        hi = min(lo + NPC, N)
        # y[128p, nsub, feat] -> node-major [nsub*128, feat]
        yp = results[i]["y"].transpose(1, 0, 2).reshape(NPC, FEAT)[: hi - lo]
        out[lo:hi] = yp[:, _INV]
    return out


def kernel(x, W0, W1, W2):
    global _COMPILED
    from concourse.bass_utils import run_bass_kernel_spmd

    if _COMPILED is None:
        _COMPILED = build_nc()
    nc = _COMPILED
    in_maps = _shard_inputs(np.asarray(x), np.asarray(W0), np.asarray(W1),
                            np.asarray(W2))
    res = run_bass_kernel_spmd(nc, in_maps, list(range(N_CORES)))
    return _unshard_output(res.results)


# revision 16
# speedup vs baseline: 1.1480x; 1.1340x over previous
"""Trainium2 Bass kernel for IrrepsLinear (128x0e + 128x1o + 128x2e).

y[n, off_l + o*d_l + d] = alpha * sum_m x[n, off_l + m*d_l + d] * W_l[m, o]

Data-parallel over nodes N across 8 cores; the whole data path runs in fp16
(fp32 accumulation in PSUM) — the harness gate is rel_err < 2e-2 and fp16
keeps it ~5e-4, while halving HBM traffic vs fp32 (the fp32 version sat at
the 358 GB/s HBM roofline).

Host-side sharding lays each core's x shard out m-major as
xg[128, 49, 9, 128] fp16: partition line m holds, for each 128-node subtile,
nine de-interleaved plane rows (one per (l, d) pair). A chunk of subtiles is
then a single contiguous DMA run per partition (up to ~16 KB descriptors).
Per subtile nine fp16 matmuls (lhsT = plane slice, rhs = alpha-scaled weight
resident in SBUF) fill two PSUM tiles: P1 [128,512] = l2 d0-3 (one bank) and
P2 [128,640] = l2 d4, l0, l1 d0-2 (two banks). One DVE copy (P1) and one ACT
copy (P2) cast fp32 -> fp16 into the plane-major SBUF output tile, which is
DMA'd to the m-major output y[128, 49, 1152]; the host inverse-permutes.
Input DMAs ride the SP HWDGE ring, output DMAs the ACT ring. Chunk sizes
taper ([3,7,...,7,3,1]) so compute starts early and the tail drains fast.
"""

import sys

sys.path.insert(0, "/opt/trn_rl_repo")

import numpy as np

N = 50000
FEAT = 1152
DIMS = [1, 3, 5]
OFFS = [0, 128, 512]
N_CORES = 8
SUB = 128            # nodes per subtile (partition dim)
NSUB = 49            # subtiles per core
NPC = NSUB * SUB     # padded nodes per core (6272)
SIZES = [3, 7, 7, 7, 7, 7, 7, 3, 1]   # subtiles per DMA unit (sum = 49)
CHMAX = max(SIZES)

# (l, d) plane order, both for the xg input and the plane-major output:
# P1 = l2 d0-3, then P2 = l1 d0-2 | l2 d4 | l0 (grouped so each W-stationary
# matmul streams a contiguous run of planes and stays within one PSUM bank).
PLANES = [(2, 0), (2, 1), (2, 2), (2, 3), (1, 0), (1, 1), (1, 2), (2, 4),
          (0, 0)]

_COMPILED = None


def build_nc(sizes=tuple(SIZES)):
    import concourse.mybir as mybir
    import concourse.tile as tile
    from concourse import bacc

    f16 = mybir.dt.float16
    f32 = mybir.dt.float32
    nsub = sum(sizes)

    nc = bacc.Bacc("TRN2", target_bir_lowering=False, debug=False,
                   num_devices=N_CORES)
    xg = nc.dram_tensor("xg", [128, nsub, 9, SUB], f16, kind="ExternalInput")
    w = nc.dram_tensor("w", [128, 3, 128], f16, kind="ExternalInput")
    y = nc.dram_tensor("y", [128, nsub, FEAT], f16, kind="ExternalOutput")

    with tile.TileContext(nc) as tc:
        with (
            tc.tile_pool(name="singles", bufs=1) as singles,
            tc.tile_pool(name="xs", bufs=4) as xpool,
            tc.tile_pool(name="ys", bufs=4) as ypool,
            tc.tile_pool(name="p1", bufs=2, space="PSUM") as p1pool,
            tc.tile_pool(name="p2", bufs=2, space="PSUM") as p2pool,
        ):
            wt = singles.tile([128, 3, 128], f16, tag="w")
            nc.sync.dma_start(out=wt, in_=w[:, :, :])
            wts = [wt[:, i, :] for i in range(3)]

            s0 = 0
            for csz in sizes:
                xt = xpool.tile([128, CHMAX, 9, SUB], f16)
                nc.sync.dma_start(out=xt[:, 0:csz], in_=xg[:, s0:s0 + csz])
                yt = ypool.tile([128, CHMAX, FEAT], f16)
                for ai in range(0, csz, 2):
                    npair = min(2, csz - ai)

                    # W-stationary matmuls: weights are the stationary
                    # operand (lhsT), x-planes stream as the moving operand,
                    # so each subtile needs only 4 matmuls (1152 streamed
                    # columns) instead of 9 — keeps the PE under the DMA
                    # cadence even when HAM throttles it to half duty.
                    # Output partitions become o (weight out-channel); the
                    # host transposes o back against nodes.
                    # P1 pair: l2 d0-3 for two subtiles (two PSUM banks) —
                    # one scalar copy per pair halves the per-instruction
                    # fixed cost on the copy engines.
                    p1 = p1pool.tile([128, 2, 512], f32, tag="p1")
                    p2s = []
                    for j in range(npair):
                        # W2 planes grouped first to minimize weight reloads
                        nc.tensor.matmul(p1[:, j, :], lhsT=wts[2],
                                         rhs=xt[:, ai + j, 0:4, :])
                        # P2: l1 d0-2 | l2 d4 | l0 (two PSUM banks; every
                        # matmul stays within a single bank)
                        p2 = p2pool.tile([128, 640], f32, tag="p2")
                        nc.tensor.matmul(p2[:, 384:512], lhsT=wts[2],
                                         rhs=xt[:, ai + j, 7, :])
                        nc.tensor.matmul(p2[:, 0:384], lhsT=wts[1],
                                         rhs=xt[:, ai + j, 4:7, :])
                        nc.tensor.matmul(p2[:, 512:640], lhsT=wts[0],
                                         rhs=xt[:, ai + j, 8, :])
                        p2s.append(p2)

                    # PSUM -> SBUF copies (fp32 -> fp16 cast), plane-major
                    # output; host undoes the column permute.
                    nc.scalar.copy(yt[:, ai:ai + npair, 0:512],
                                   p1[:, 0:npair])
                    for j in range(npair):
                        nc.vector.tensor_copy(yt[:, ai + j, 512:1152],
                                              p2s[j])
                # output DMA on the ACT HWDGE ring (separate FIFO from inputs)
                nc.scalar.dma_start(out=y[:, s0:s0 + csz], in_=yt[:, 0:csz])
                s0 += csz

    nc.compile()
    return nc


# plane q row m <- original feature column off_l + m*d_l + d; also the
# output-side permutation (plane-major column q*128+o -> natural column).
_PERM = np.concatenate([
    np.arange(128) * DIMS[l] + OFFS[l] + d for (l, d) in PLANES
])
_INV = np.empty(FEAT, np.int64)
_INV[_PERM] = np.arange(FEAT)


def _shard_inputs(x, W0, W1, W2):
    alpha = np.float32(1.0 / np.sqrt(128.0))
    ws = {"w": np.ascontiguousarray(
        np.stack([W0 * alpha, W1 * alpha, W2 * alpha], axis=1),
        dtype=np.float16)}
    x16 = np.asarray(x, dtype=np.float16)
    in_maps = []
    for i in range(N_CORES):
        lo = i * NPC
        hi = min(lo + NPC, N)
        xs = x16[lo:hi]
        xp = np.empty((9 * 128, NPC), np.float16)
        xp[:, : hi - lo] = xs.T[_PERM]
        if hi - lo < NPC:
            xp[:, hi - lo:] = 0.0
        # [9, 128m, nsub, 128n] -> m-major [128m, nsub, 9, 128n]
        xg = np.ascontiguousarray(
            xp.reshape(9, 128, NSUB, SUB).transpose(1, 2, 0, 3))
        in_maps.append({"xg": xg, **ws})
    return in_maps


def _unshard_output(results):
    out = np.empty((N, FEAT), np.float32)
    for i in range(N_CORES):
        lo = i * NPC
        hi = min(lo + NPC, N)
        # y[128o, nsub, (q,n)] -> node-major [(s,n), (q,o)]
        yp = results[i]["y"].reshape(128, NSUB, 9, SUB).transpose(
            1, 3, 2, 0).reshape(NPC, FEAT)[: hi - lo]
        out[lo:hi] = yp[:, _INV]
    return out


def kernel(x, W0, W1, W2):
    global _COMPILED
    from concourse.bass_utils import run_bass_kernel_spmd

    if _COMPILED is None:
        _COMPILED = build_nc()
    nc = _COMPILED
    in_maps = _shard_inputs(np.asarray(x), np.asarray(W0), np.asarray(W1),
                            np.asarray(W2))
    res = run_bass_kernel_spmd(nc, in_maps, list(range(N_CORES)))
    return _unshard_output(res.results)
